# revision 1
# baseline (speedup 1.0000x reference)
"""KAN layer (B=8192, IN_F=OUT_F=1024, GRID=5) on 8 Trainium2 cores.

Math: Y[b,o] = W0[o]*silu(x) + W1[o]*spline_o(clip(x,-1,1)) + b[o], x = X[b,o]
(idx_in = arange(O) % IN_F is the identity here since O == IN_F).

The degree-1 B-spline on the uniform 5-knot grid over [-1,1] is rewritten in
the relu basis: spline(xc) = c0 + m0*(xc+1) + sum_j (m_j - m_{j-1})*relu(xc - s_j)
with slopes m_g = 2*(c_{g+1}-c_g) and interior knots s_j in {-0.5, 0, 0.5}.
Folding W1 and b gives  Y^T[o,:] = W0*silu + B'*xc + G1*r1 + G2*r2 + G3*r3 + A'.

Layout: edges on SBUF partitions (X pre-transposed on host), batch on the free
dim, data-parallel over batch across the 8 cores.  The per-edge weighted sum of
the 5 feature maps runs on TensorE as 5 diagonal-stationary matmuls (silu in
fp32r, the four spline features in fp16) accumulating in PSUM; ScalarE
evacuates PSUM adding the per-edge bias A'.  Diagonal stationaries are built
on-device (identity * per-partition weight).  DMA: per-block input loads on the
Sync HWDGE queue, output stores on GpSimd SWDGE — ScalarE issues no DMAs.
"""
import sys

for _p in ("/root/.axon_site", "/root/.axon_site/_ro/trn_rl_repo", "/root/.axon_site/_ro/pypackages"):
    if _p not in sys.path:
        sys.path.append(_p)

import numpy as np

import concourse.bacc as bacc
import concourse.tile as tile
from concourse import mybir
from concourse.bass_utils import run_bass_kernel_spmd

B, IN_F, OUT_F, GRID = 8192, 1024, 1024, 5
N_CORES = 8
B_SHARD = B // N_CORES          # 1024 batch rows per core
EB = OUT_F // 128               # 8 edge blocks
NF = 5                          # features: silu, xc, r1, r2, r3
CHUNK = 512                     # one PSUM bank of fp32

# cpack layout (fp32 columns): [0:128] identity, [128:168] wT (5 weights x 8
# blocks, feature-major per block), [168:176] A'
WOFF, AOFF, CCOLS = 128, 168, 176

_nc_cache = None


def _build():
    f32 = mybir.dt.float32
    f32r = mybir.dt.float32r
    f16 = mybir.dt.float16
    AF = mybir.ActivationFunctionType
    OP = mybir.AluOpType
    nc = bacc.Bacc("TRN2", target_bir_lowering=False, debug=False)
    xt = nc.dram_tensor("xt", [OUT_F, B_SHARD], f32, kind="ExternalInput").ap()
    cpack = nc.dram_tensor("cpack", [128, CCOLS], f32, kind="ExternalInput").ap()
    yt = nc.dram_tensor("yt", [OUT_F, B_SHARD], f32, kind="ExternalOutput").ap()

    xt3 = xt.rearrange("(n p) d -> p n d", p=128)   # [128, EB, B_SHARD]
    yt3 = yt.rearrange("(n p) d -> p n d", p=128)

    with tile.TileContext(nc) as tc:
        with tc.tile_pool(name="const", bufs=1) as const_pool, \
             tc.tile_pool(name="xin", bufs=4) as xin_pool, \
             tc.tile_pool(name="feat", bufs=3) as feat_pool, \
             tc.tile_pool(name="feat0", bufs=1) as feat0_pool, \
             tc.tile_pool(name="yout", bufs=3) as yout_pool, \
             tc.tile_pool(name="ps", bufs=3, space="PSUM") as psum_pool, \
             tc.tile_pool(name="pswarm", bufs=1, space="PSUM") as warm_pool:
            cp = const_pool.tile([128, CCOLS], f32)
            nc.sync.dma_start(cp[:], cpack[:, :])
            ident32 = cp[:, 0:128]
            wv = cp[:, WOFF:WOFF + 40]               # [128, 40] fp32 weights
            ident16 = const_pool.tile([128, 128], f16)
            nc.vector.tensor_copy(ident16[:], ident32)

            # HAM warm-up: ~4.5us of dummy matmuls on uninitialized SBUF so
            # the PE clock-gate opens before the first real matmul arrives
            scratch = const_pool.tile([128, CHUNK], f16)
            nc.vector.memset(scratch[:], 0.0)
            ps_warm = warm_pool.tile([128, CHUNK], f32, tag="pswarm", name="pswarm")
            for _ in range(9):
                nc.tensor.matmul(ps_warm[:], scratch[:, 0:128], scratch[:],
                                 start=True, stop=True, skip_group_check=True)

            # per-block diagonal stationaries, built on device (emitted inside
            # the block loop so the pipeline starts immediately)
            dsilu = const_pool.tile([128, EB * 128], f32r)
            dspl = const_pool.tile([128, EB * 4 * 128], f16)

            def feature_ops(xv, n, tagsuf):
                """xv: [128, n, B_SHARD] input view -> 5 feature tiles."""
                pool = feat0_pool if tagsuf else feat_pool
                silu_t = pool.tile([128, n, B_SHARD], f32r, tag="silu" + tagsuf,
                                        name=f"silu{tagsuf}")
                nc.scalar.activation(silu_t[:], xv, AF.Silu)
                xc_t = pool.tile([128, n, B_SHARD], f16, tag="xc" + tagsuf,
                                      name=f"xc{tagsuf}")
                nc.vector.tensor_scalar(xc_t[:], xv, 1.0, -1.0, OP.min, OP.max)
                r1_t = pool.tile([128, n, B_SHARD], f16, tag="r1" + tagsuf,
                                      name=f"r1{tagsuf}")
                nc.vector.tensor_scalar(r1_t[:], xc_t[:], 0.5, 0.0, OP.add, OP.max)
                r2_t = pool.tile([128, n, B_SHARD], f16, tag="r2" + tagsuf,
                                      name=f"r2{tagsuf}")
                nc.vector.tensor_scalar_max(r2_t[:], xc_t[:], 0.0)
                r3_t = pool.tile([128, n, B_SHARD], f16, tag="r3" + tagsuf,
                                      name=f"r3{tagsuf}")
                nc.vector.tensor_scalar(r3_t[:], xc_t[:], -0.5, 0.0, OP.add, OP.max)
                return silu_t, xc_t, r1_t, r2_t, r3_t

            def block_matmuls(e, feats, hh, yo):
                """Build diags for block e, run the 10 matmuls, evacuate."""
                silu_t, xc_t, r1_t, r2_t, r3_t = feats
                ds = dsilu[:, e * 128:(e + 1) * 128]
                nc.vector.tensor_scalar_mul(ds, ident32, wv[:, e * NF:e * NF + 1])
                for j in range(4):
                    nc.vector.tensor_scalar_mul(
                        dspl[:, (e * 4 + j) * 128:(e * 4 + j + 1) * 128],
                        ident16[:], wv[:, e * NF + 1 + j:e * NF + 2 + j])
                ps = psum_pool.tile([128, B_SHARD], f32, tag="ps", name=f"ps_{e}")

                # xc is ready before silu (clip is cheaper than the ACT pass),
                # so start each block's accumulation with the spline features
                # and finish with silu
                def block_chunk(ts):
                    for j, ft in enumerate((xc_t, r1_t, r2_t, r3_t)):
                        for t in ts:
                            nc.tensor.matmul(ps[:, t * CHUNK:(t + 1) * CHUNK],
                                             dspl[:, (e * 4 + j) * 128:(e * 4 + j + 1) * 128],
                                             ft[:, hh, t * CHUNK:(t + 1) * CHUNK],
                                             start=(j == 0), stop=False,
                                             skip_group_check=True)
                    for t in ts:
                        nc.tensor.matmul(ps[:, t * CHUNK:(t + 1) * CHUNK], ds,
                                         silu_t[:, hh, t * CHUNK:(t + 1) * CHUNK],
                                         start=False, stop=True, skip_group_check=True)

                if e < EB - 1:
                    block_chunk((0, 1))
                    nc.scalar.activation(yo[:, e % 2, :], ps[:], AF.Identity,
                                         bias=cp[:, AOFF + e:AOFF + e + 1], scale=1.0)
                else:
                    # last block: per-chunk pipeline on VectorE for a short tail
                    for t in range(2):
                        block_chunk((t,))
                        nc.vector.tensor_scalar_add(
                            yo[:, e % 2, t * CHUNK:(t + 1) * CHUNK],
                            ps[:, t * CHUNK:(t + 1) * CHUNK],
                            cp[:, AOFF + e:AOFF + e + 1])

            for ep in range(EB // 2):
                if ep == 0:
                    # first pair: per-block DMAs and per-block features so
                    # compute starts as soon as 512 KB has landed
                    yo = yout_pool.tile([128, 2, B_SHARD], f32, tag="yo", name="yo_p0")
                    for h in range(2):
                        x_t = xin_pool.tile([128, 1, B_SHARD], f32, tag=f"x0{h}",
                                            name=f"x0{h}")
                        nc.sync.dma_start(x_t[:], xt3[:, h:h + 1, :])
                        feats = feature_ops(x_t[:], 1, f"0{h}")
                        block_matmuls(h, feats, 0, yo)
                else:
                    x_t = xin_pool.tile([128, 2, B_SHARD], f32, tag="x",
                                        name=f"x_p{ep}")
                    nc.sync.dma_start(x_t[:], xt3[:, 2 * ep:2 * ep + 2, :])
                    feats = feature_ops(x_t[:], 2, "")
                    yo = yout_pool.tile([128, 2, B_SHARD], f32, tag="yo",
                                        name=f"yo_p{ep}")
                    for h in range(2):
                        block_matmuls(2 * ep + h, feats, h, yo)
                if ep == EB // 2 - 1:
                    # split the last stores across two queues for a short tail
                    nc.gpsimd.dma_start(yt3[:, 2 * ep:2 * ep + 1, :], yo[:, 0:1, :])
                    nc.gpsimd.dma_start(yt3[:, 2 * ep + 1:2 * ep + 2, 0:CHUNK],
                                        yo[:, 1:2, 0:CHUNK])
                    nc.sync.dma_start(yt3[:, 2 * ep + 1:2 * ep + 2, CHUNK:B_SHARD],
                                      yo[:, 1:2, CHUNK:B_SHARD])
                elif ep % 2 == 0:
                    nc.gpsimd.dma_start(yt3[:, 2 * ep:2 * ep + 2, :], yo[:])
                else:
                    nc.sync.dma_start(yt3[:, 2 * ep:2 * ep + 2, :], yo[:])
    nc.compile()
    return nc


def _host_prep(X, coeffs, W, b):
    c = coeffs.astype(np.float64)
    W = W.astype(np.float64)
    b = b.astype(np.float64)
    m = 2.0 * (c[:, 1:] - c[:, :-1])            # [O, 4] slopes per unit xc
    w1 = W[:, 1]
    aprime = w1 * (c[:, 0] + m[:, 0]) + b        # const term (incl. m0*(xc+1) fold)
    bprime = w1 * m[:, 0]
    g = w1[:, None] * (m[:, 1:] - m[:, :-1])     # [O, 3] relu weights at s=-0.5,0,0.5
    wvec = np.stack([W[:, 0], bprime, g[:, 0], g[:, 1], g[:, 2]], axis=1)  # [O, 5]

    cpack = np.zeros((128, CCOLS), dtype=np.float32)
    cpack[:, 0:128] = np.eye(128, dtype=np.float32)
    for e in range(EB):
        for f in range(NF):
            cpack[:, WOFF + e * NF + f] = wvec[e * 128:(e + 1) * 128, f].astype(np.float32)
        cpack[:, AOFF + e] = aprime[e * 128:(e + 1) * 128].astype(np.float32)
    return cpack


def kernel(X, coeffs, W, b):
    global _nc_cache
    if _nc_cache is None:
        _nc_cache = _build()
    nc = _nc_cache

    cpack = _host_prep(X, coeffs, W, b)
    in_maps = []
    for c in range(N_CORES):
        xt_shard = np.ascontiguousarray(X[c * B_SHARD:(c + 1) * B_SHARD, :].T)
        in_maps.append({"xt": xt_shard, "cpack": cpack})

    res = run_bass_kernel_spmd(nc, in_maps, core_ids=list(range(N_CORES)))
    Y = np.empty((B, OUT_F), dtype=np.float32)
    for c in range(N_CORES):
        Y[c * B_SHARD:(c + 1) * B_SHARD, :] = res.results[c]["yt"].T.astype(np.float32)
    return Y



# revision 2
# speedup vs baseline: 1.0276x; 1.0276x over previous
"""KAN layer (B=8192, IN_F=OUT_F=1024, GRID=5) on 8 Trainium2 cores.

Math: Y[b,o] = W0[o]*silu(x) + spline_o(clip(x,-1,1)) * W1[o] + b[o], x = X[b,o]
(idx_in = arange(O) % IN_F is the identity here since O == IN_F).

Factorization used here (clip-form basis, exact):
  Y = W0*silu(x) + B'*xc + G1*M1 + G2*M2 + G3*M3 + A''
  xc  = clip(x, -1, 1)
  Mj  = clip(x, s_j, 1),  s_j in {-0.5, 0.0, 0.5}
  B'  = w1*sl0;  Gj = w1*(sl_j - sl_{j-1});  sl_g = 2*(c_{g+1}-c_g)
  A'' = w1*(c0 + sl0 + 0.5*d1 - 0.5*d3) + b   (d_j = sl_j - sl_{j-1})

Sharding: EDGES across the 8 cores (128 edges per core, full batch 8192 on
the free dim).  Per core only 5 diagonal stationaries are needed; X arrives
pre-transposed and cast to fp16 on host ([128 edges, 8192 batch]), output
returns as fp16 and is cast back on host.  This halves DMA traffic in both
directions (tolerance is 2e-2; fp16 I/O costs ~1e-3).

Per 512-col chunk the per-edge weighted sum runs on TensorE as diagonal
fp16 matmuls accumulating in PSUM.  Even chunks skip the M3 matmul: their
evacuation runs on DVE as affine_then_add (yo = M3*G3 + A'' + psum), odd
chunks evacuate on ScalarE (Identity + A'' bias) after a 5th matmul.  silu
on ScalarE, xc/M1/M3 on DVE (tensor_scalar, 4x fp16 mode), M2 on GpSimd.
All DMA on the Sync (SP) HWDGE queue; inputs are fully prefetched.
"""
import sys

for _p in ("/root/.axon_site", "/root/.axon_site/_ro/trn_rl_repo", "/root/.axon_site/_ro/pypackages"):
    if _p not in sys.path:
        sys.path.append(_p)

import numpy as np

import concourse.bacc as bacc
import concourse.tile as tile
from concourse import mybir
from concourse.bass_utils import run_bass_kernel_spmd

B, IN_F, OUT_F, GRID = 8192, 1024, 1024, 5
N_CORES = 8
E_SHARD = OUT_F // N_CORES      # 128 edges per core
NG = 8                          # batch groups of 1024
GW = B // NG                    # group width (1024)
CHUNK = 512                     # one PSUM bank of fp32
N_WARM = 6                      # PE clock-gate warmup matmuls
M2_ON_POOL = True               # compute M2 on GpSimd (else DVE)

_nc_cache = None


def _build():
    f16 = mybir.dt.float16
    f32 = mybir.dt.float32
    AF = mybir.ActivationFunctionType
    OP = mybir.AluOpType
    nc = bacc.Bacc("TRN2", target_bir_lowering=False, debug=False)
    xt = nc.dram_tensor("xt", [E_SHARD, B], f16, kind="ExternalInput").ap()
    wp = nc.dram_tensor("wp", [E_SHARD, 8], f32, kind="ExternalInput").ap()
    idn = nc.dram_tensor("idn", [E_SHARD, E_SHARD], f16, kind="ExternalInput").ap()
    yt = nc.dram_tensor("yt", [E_SHARD, B], f16, kind="ExternalOutput").ap()

    with tile.TileContext(nc) as tc:
        with tc.tile_pool(name="const", bufs=1) as cpool, \
             tc.tile_pool(name="xin", bufs=NG) as xpool, \
             tc.tile_pool(name="fsil", bufs=3) as spool, \
             tc.tile_pool(name="fxc", bufs=3) as xcpool, \
             tc.tile_pool(name="fm1", bufs=3) as m1pool, \
             tc.tile_pool(name="fm2", bufs=3) as m2pool, \
             tc.tile_pool(name="fm3", bufs=3) as m3pool, \
             tc.tile_pool(name="yout", bufs=3) as ypool, \
             tc.tile_pool(name="ps", bufs=7, space="PSUM") as pspool, \
             tc.tile_pool(name="pswarm", bufs=1, space="PSUM") as wpool:

            # --- warmups: no data deps, start the PE clock ramp and the
            # ACT table load while the first DMAs are in flight
            scr = cpool.tile([128, CHUNK], f16)
            ps_warm = wpool.tile([128, CHUNK], f32, tag="pswarm", name="pswarm")
            for _ in range(N_WARM):
                nc.tensor.matmul(ps_warm[:], scr[:, 0:128], scr[:],
                                 start=True, stop=True, skip_group_check=True)
            nc.scalar.activation(scr[:, 0:1], scr[:, 0:1], AF.Silu)
            nc.scalar.activation(scr[:, 1:2], scr[:, 1:2], AF.Identity)

            # --- constants
            wpt = cpool.tile([128, 8], f32)
            nc.sync.dma_start(wpt[:], wp[:, :])
            idt = cpool.tile([128, 128], f16)
            nc.sync.dma_start(idt[:], idn[:, :])

            # --- full input prefetch (8 x 256KB on the SP queue)
            xg = []
            for g in range(NG):
                x_t = xpool.tile([128, GW], f16, tag=f"x{g}", name=f"x{g}")
                nc.sync.dma_start(x_t[:], xt[:, g * GW:(g + 1) * GW])
                xg.append(x_t)

            # --- diagonal stationaries: diag[f] = idn * w_f (per-partition)
            # order: 0=W0(silu) 1=B'(xc) 2=G1(M1) 3=G2(M2) 4=G3(M3)
            diag = cpool.tile([128, 5, 128], f16)
            for f in range(5):
                nc.vector.tensor_scalar_mul(diag[:, f, :], idt[:], wpt[:, f:f + 1])
            g3c = wpt[:, 4:5]
            apc = wpt[:, 5:6]

            for g in range(NG):
                xv = xg[g][:]
                sil = spool.tile([128, GW], f16, tag="sil", name=f"sil{g}")
                nc.scalar.activation(sil[:], xv, AF.Silu)
                xc = xcpool.tile([128, GW], f16, tag="xc", name=f"xc{g}")
                nc.vector.tensor_scalar(xc[:], xv, 1.0, -1.0, OP.min, OP.max)
                m1 = m1pool.tile([128, GW], f16, tag="m1", name=f"m1{g}")
                nc.vector.tensor_scalar(m1[:], xv, 1.0, -0.5, OP.min, OP.max)
                m2 = m2pool.tile([128, GW], f16, tag="m2", name=f"m2{g}")
                if M2_ON_POOL:
                    nc.gpsimd.tensor_scalar(m2[:], xv, 1.0, 0.0, OP.min, OP.max)
                else:
                    nc.vector.tensor_scalar(m2[:], xv, 1.0, 0.0, OP.min, OP.max)
                m3 = m3pool.tile([128, GW], f16, tag="m3", name=f"m3{g}")
                nc.vector.tensor_scalar(m3[:], xv, 1.0, 0.5, OP.min, OP.max)

                yo = ypool.tile([128, GW], f16, tag="yo", name=f"yo{g}")
                for h in range(2):
                    cs = slice(h * CHUNK, (h + 1) * CHUNK)
                    ps = pspool.tile([128, CHUNK], f32, tag="ps", name=f"ps{g}_{h}")
                    nc.tensor.matmul(ps[:], diag[:, 1, :], xc[:, cs],
                                     start=True, stop=False, skip_group_check=True)
                    nc.tensor.matmul(ps[:], diag[:, 2, :], m1[:, cs],
                                     start=False, stop=False, skip_group_check=True)
                    nc.tensor.matmul(ps[:], diag[:, 3, :], m2[:, cs],
                                     start=False, stop=False, skip_group_check=True)
                    if h == 0:
                        # even chunk: M3*G3 + A'' folded into the DVE evac
                        nc.tensor.matmul(ps[:], diag[:, 0, :], sil[:, cs],
                                         start=False, stop=True,
                                         skip_group_check=True)
                        nc.vector.affine_then_add(yo[:, cs], m3[:, cs], ps[:],
                                                  scale=g3c, bias=apc)
                    else:
                        nc.tensor.matmul(ps[:], diag[:, 4, :], m3[:, cs],
                                         start=False, stop=False,
                                         skip_group_check=True)
                        nc.tensor.matmul(ps[:], diag[:, 0, :], sil[:, cs],
                                         start=False, stop=True,
                                         skip_group_check=True)
                        nc.scalar.activation(yo[:, cs], ps[:], AF.Identity,
                                             bias=apc, scale=1.0)
                nc.sync.dma_start(yt[:, g * GW:(g + 1) * GW], yo[:])
    nc.compile()
    return nc


def _host_prep(X, coeffs, W, b):
    c = coeffs.astype(np.float64)
    Wd = W.astype(np.float64)
    bd = b.astype(np.float64)
    sl = 2.0 * (c[:, 1:] - c[:, :-1])           # [O, 4] segment slopes
    d = sl[:, 1:] - sl[:, :-1]                  # [O, 3] slope deltas at knots
    w1 = Wd[:, 1]
    bprime = w1 * sl[:, 0]
    g = w1[:, None] * d                         # [O, 3]
    app = w1 * (c[:, 0] + sl[:, 0] + 0.5 * d[:, 0] - 0.5 * d[:, 2]) + bd

    wp = np.zeros((OUT_F, 8), dtype=np.float32)
    wp[:, 0] = Wd[:, 0]
    wp[:, 1] = bprime
    wp[:, 2] = g[:, 0]
    wp[:, 3] = g[:, 1]
    wp[:, 4] = g[:, 2]
    wp[:, 5] = app
    idn = np.eye(E_SHARD, dtype=np.float16)
    return wp, idn


def _in_maps(X, coeffs, W, b):
    wp, idn = _host_prep(X, coeffs, W, b)
    in_maps = []
    for c in range(N_CORES):
        sl = slice(c * E_SHARD, (c + 1) * E_SHARD)
        xt = np.ascontiguousarray(X[:, sl].T.astype(np.float16))
        in_maps.append({"xt": xt, "wp": np.ascontiguousarray(wp[sl]), "idn": idn})
    return in_maps


def kernel(X, coeffs, W, b):
    global _nc_cache
    if _nc_cache is None:
        _nc_cache = _build()
    nc = _nc_cache

    in_maps = _in_maps(X, coeffs, W, b)
    res = run_bass_kernel_spmd(nc, in_maps, core_ids=list(range(N_CORES)))
    Y = np.empty((B, OUT_F), dtype=np.float32)
    for c in range(N_CORES):
        sl = slice(c * E_SHARD, (c + 1) * E_SHARD)
        Y[:, sl] = res.results[c]["yt"].T.astype(np.float32)
    return Y
